# revision 35
# baseline (speedup 1.0000x reference)
"""Mask R-CNN paste_masks_in_image on Trainium2 (Bass/Tile), 8-core data-parallel.

Per image: 16 boxes pasted sequentially (overwrite semantics) onto a 1024x1024
canvas; output = canvas*2-1 with background -1.

Strategy (v3: exponent-priority max-compositing)
------------------------------------------------
Host computes, per box k (paste order), indicator-GATED interpolation
matrices so one PE matmul per 128-row tile produces
    word = (1.25 + bilin/2) * 2^k   inside the box,   EXACTLY 0 outside.
The per-k fp16 value ranges [1.25*2^k, 1.75*2^k] are disjoint, so a plain
fp16 tensor_tensor MAX over boxes implements overwrite-by-paste-order with
no ordering dependencies, no predication, and DVE's 2x_1p perf mode.

Per box: 3 matmuls (fp16 lhsT [32,384-row-window] x rhs [32,226-col-window]
-> PSUM [128,3,226]), one ACT Copy drains PSUM f32 -> SBUF fp16 (the only
way out of PSUM), one DVE TT-max into the fp16 canvas at a register-dynamic
(row-tile, col) window. Per image: decode word -> val with two 4x DVE
tensor_scalar ops (mask mantissa + force exponent to 4.0 via bit ops on the
u16 view, then subtract 6), stores go HWDGE(sync) + SWDGE(gpsimd) as fp16
(the host casts to f32 - halves store traffic). Pool only memsets canvases
(u32-bitcast trick); gpsimd compute ucode (tensor_scalar etc.) is 10-20x
slower than the cost model claims - keep real work off it.

Baseline (ACT relu + copy_predicated serial chain): 104 us. This version:
~78 us per 4-image pipeline.
"""

import numpy as np

import concourse.bass as bass
import concourse.bacc as bacc
import concourse.mybir as mybir
import concourse.tile as tile
from concourse.bass_utils import run_bass_kernel_spmd

F32 = mybir.dt.float32
F16 = mybir.dt.float16
I32 = mybir.dt.int32
U32 = mybir.dt.uint32

B, N, M, H, W = 32, 16, 28, 1024, 1024
MP = M + 2          # padded mask size, 30
NCORES = 8
IMGS = B // NCORES  # images per core, 4
NBOX = IMGS * N     # boxes per core, 64
KDIM = 32           # indicator row + 30 gated hat rows + zero pad row
RWIN = 384          # row window: 3 row-tiles of 128
CWIN = 226          # col window (max box width 217)
TMAX = H // 128 - RWIN // 128   # max row-tile start, 5
CMAX = W - CWIN                 # max col window start, 798
GS = 3                          # boxes per 96-partition group (PE base 0/32/64)
GROUPS = 6                      # groups per image (ceil(16/3))
PCOLS = 256                     # psum plane stride (bank alignment)
FP16_BG_PAIR = 0x3D003D00       # two packed fp16 1.25s (background word)


def _host_prep(masks, rects):
    bn = B * N
    mm = np.asarray(masks, np.float32).reshape(bn, M, M)
    m_pad = np.zeros((bn, MP, MP), np.float64)
    m_pad[:, 1:-1, 1:-1] = (mm.astype(np.float64) + 1.0) * 0.5

    r = np.asarray(rects, np.float32).reshape(bn, 4)
    x0, y0, x1, y1 = r[:, 0], r[:, 1], r[:, 2], r[:, 3]
    # float32 ops in the reference's exact order (trunc boundaries must match)
    half = np.float32(0.5 * (float(MP) / M))
    w_half = (x1 - x0) * half
    h_half = (y1 - y0) * half
    x_c = (x1 + x0) * np.float32(0.5)
    y_c = (y1 + y0) * np.float32(0.5)
    b0 = np.trunc(x_c - w_half).astype(np.int32)   # row start
    b1 = np.trunc(y_c - h_half).astype(np.int32)   # col start
    b2 = np.trunc(x_c + w_half).astype(np.int32)   # row end (incl)
    b3 = np.trunc(y_c + h_half).astype(np.int32)   # col end (incl)

    # per-slot exact sizing: sort each image's boxes by (row-span, width)
    # descending; slot s takes the element-wise max over the 8 cores of the
    # s-th sorted box's span and width, so every box fits its slot by
    # construction (compositing is order-free - priority lives in the value
    # encoding).
    first = np.clip(b0 // 128, 0, 7)
    last = np.clip(np.clip(b2, 0, H - 1) // 128, 0, 7)
    span = np.clip(last - first + 1, 1, 3)
    wbox = np.clip(b3 - b1 + 1, 1, W)
    key = (span * 1024 + np.minimum(wbox, 1023)).reshape(B, N)
    perm = np.argsort(-key, axis=1, kind="stable")
    flat_perm = (perm + np.arange(B)[:, None] * N).reshape(bn)
    span_s = span[flat_perm].reshape(NCORES, IMGS, N).max(axis=0)   # [4,16]
    w_s = wbox[flat_perm].reshape(NCORES, IMGS, N).max(axis=0)
    win_s = np.minimum(CWIN, (w_s + 2) & ~1)                        # even cols
    mixes = (tuple(map(tuple, span_s.tolist())),
             tuple(map(tuple, win_s.tolist())))

    # exponent-priority factor by ORIGINAL paste index, then permute all
    # per-box arrays into slot order
    p2k = np.exp2(np.tile(np.arange(N, dtype=np.float64), B))
    b0, b1, b2, b3 = (a[flat_perm] for a in (b0, b1, b2, b3))
    first = first[flat_perm]
    m_pad = m_pad[flat_perm]
    p2k = p2k[flat_perm]
    hgt = np.maximum(b2 - b0 + 1, 1).astype(np.float64)
    wid = np.maximum(b3 - b1 + 1, 1).astype(np.float64)

    # slot-aware window clips: t0 <= 8 - span_slot; c0 <= W - win_slot
    pos = (np.arange(bn) // N) % IMGS
    slot = np.tile(np.arange(N), B)
    sp_of = span_s[pos, slot]
    win_of = win_s[pos, slot]
    t0 = np.minimum(np.clip(first, 0, 7), 8 - sp_of).astype(np.int32)
    # even col starts keep the fp16 canvas writes 4B-aligned (DVE write port)
    c0 = (np.minimum(np.clip(b1, 0, W), W - win_of) & ~1).astype(np.int32)

    i_idx = np.arange(MP, dtype=np.float64)

    p = np.arange(RWIN, dtype=np.float64)
    g = t0[:, None].astype(np.float64) * 128 + p[None, :]          # [bn, 384]
    sx = (g - b0[:, None] + 0.5) * (MP / hgt)[:, None] - 0.5
    sx = np.clip(sx, 0.0, MP - 1.0)
    rx = np.maximum(0.0, 1.0 - np.abs(sx[:, None, :] - i_idx[None, :, None]))
    in_row = ((g >= b0[:, None]) & (g <= b2[:, None])).astype(np.float64)

    lhsT = np.zeros((bn, KDIM, RWIN), np.float16)
    lhsT[:, 0, :] = in_row
    lhsT[:, 1:MP + 1, :] = rx * in_row[:, None, :]

    q = np.arange(CWIN, dtype=np.float64)
    gc = c0[:, None].astype(np.float64) + q[None, :]               # [bn, 226]
    sy = (gc - b1[:, None] + 0.5) * (MP / wid)[:, None] - 0.5
    sy = np.clip(sy, 0.0, MP - 1.0)
    ry = np.maximum(0.0, 1.0 - np.abs(sy[:, None, :] - i_idx[None, :, None]))
    mry = 2.0 * np.einsum('bij,bjq->biq', m_pad, ry)
    in_col = ((gc >= b1[:, None]) & (gc <= b3[:, None])).astype(np.float64)

    rhs = np.zeros((bn, KDIM, CWIN), np.float16)
    rhs[:, 0, :] = in_col * (1.25 * p2k)[:, None]
    rhs[:, 1:MP + 1, :] = (mry * in_col[:, None, :]) * (0.25 * p2k)[:, None, None]

    boxdata = np.concatenate([lhsT, rhs], axis=2)   # [bn, 32, 610]
    trip = np.stack([t0, c0], axis=1).astype(np.int32)    # [bn, 2]
    # PE matmul sources must start at partition 0/32/64, so pack 3 boxes per
    # 96-partition group; 16 boxes/image pad to 18 slots (6 groups).
    bd = boxdata.reshape(B, N, KDIM, RWIN + CWIN)
    pad = np.zeros((B, 2, KDIM, RWIN + CWIN), np.float16)
    bd = np.concatenate([bd, pad], axis=1)          # [B, 18, 32, 610]
    bd = bd.reshape(B * GROUPS, GS * KDIM, RWIN + CWIN)   # [B*6, 96, 610]
    return bd, trip, mixes


def build_nc(loop_reps=1, decode_eng="dve", store="cast", probe="", mixes=None):
    # probe: comma-set of {nostore,nocopy,nodrain,nodecode,nomm} — timing-only
    # ablations that skip stages (output becomes garbage)
    probes = set(probe.split(",")) if probe else set()
    if mixes is None:
        mixes = _MIXES[0] if _MIXES else (((3,) * N,) * IMGS,
                                          ((CWIN,) * N,) * IMGS)
    span_mix, win_mix = mixes
    # Bacc defers register allocation to a graph-coloring pass, which the
    # per-box dynamic canvas offsets need (raw Bass exhausts the register
    # pool). loop_reps > 1 wraps the pipeline in a device-side For_i so
    # wall-clock slope measurements can resolve the ~us-scale kernel time.
    nc = bacc.Bacc()
    boxdata_d = nc.declare_dram_parameter(
        "boxdata", [IMGS * GROUPS, GS * KDIM, RWIN + CWIN], F16, isOutput=False)
    tcoff_d = nc.declare_dram_parameter("tcoff", [1, 2 * NBOX], I32, isOutput=False)
    out_d = nc.declare_dram_parameter("out", [IMGS, H, W], F16, isOutput=True)
    DVE_E = mybir.EngineType.DVE

    with tile.TileContext(nc) as tc:
        with (
            tc.tile_pool(name="canvas", bufs=4) as canvas_pool,
            tc.tile_pool(name="boxes", bufs=2) as box_pool,
            tc.tile_pool(name="msk", bufs=10) as mask_pool,
            tc.tile_pool(name="offs", bufs=1) as offs_pool,
            tc.tile_pool(name="psum", bufs=4, space=bass.MemorySpace.PSUM) as psum_pool,
        ):
            U16 = mybir.dt.uint16
            tc_sb = offs_pool.tile([1, 2 * NBOX], I32, tag="tcoff")
            nc.sync.dma_start(tc_sb[:], tcoff_d[:])

            def pipeline():
                canvases = []
                # all four canvas memsets up front on Pool (cheap via the
                # u32-bitcast trick) so DVE images never wait mid-stream
                for img in range(IMGS):
                    cv = canvas_pool.tile([128, H // 128, W], F16, tag="canvas")
                    canvases.append(cv)
                    nc.gpsimd.memset(cv[:].bitcast(U32), FP16_BG_PAIR)
                regs = {}
                for img in range(IMGS):
                    ceng = nc.vector
                    eng_t = DVE_E
                    canvas = canvases[img]
                    # one batched offset-register load per image
                    base = img * N
                    batch = []
                    for bm in range(base, base + N):
                        regs[bm] = tuple(
                            nc.alloc_register(eng_t, f"{nm}{bm}")
                            for nm in ("t", "c"))
                        batch.extend(regs[bm])
                    nc.reg_load(batch, tc_sb[0:1, 2 * base:2 * (base + N)])
                    # two strided DMAs load all 16 boxes' matrices, 3 boxes
                    # packed per 96 partitions
                    bdi = box_pool.tile([GS * KDIM, GROUPS, RWIN + CWIN], F16,
                                        tag="bdi")
                    src = boxdata_d[img * GROUPS:(img + 1) * GROUPS].rearrange(
                        "g k c -> k g c")
                    half = GROUPS // 2
                    nc.sync.dma_start(bdi[:, 0:half, :], src[:, 0:half, :])
                    nc.sync.dma_start(bdi[:, half:GROUPS, :], src[:, half:GROUPS, :])
                    for n in range(N):
                        bi = img * N + n
                        j, g2 = n % GS, n // GS
                        p0 = KDIM * j
                        sp = span_mix[img][n]
                        cw = win_mix[img][n]
                        cmax = W - cw
                        ps = psum_pool.tile([128, 3, PCOLS], F32, tag="ps")
                        m = mask_pool.tile([128, 3, CWIN], F16, tag="m")
                        rhs_ap = bdi[p0:p0 + KDIM, g2, RWIN:RWIN + cw]
                        if "nomm" not in probes:
                            for k in range(sp):
                                nc.tensor.matmul(
                                    ps[:, k, 0:cw],
                                    bdi[p0:p0 + KDIM, g2, k * 128:(k + 1) * 128],
                                    rhs_ap,
                                    start=True, stop=True,
                                )
                        else:
                            nc.vector.memset(ps[:, 0:1, 0:8], 0.0)
                        if "nodrain" not in probes:
                            nc.scalar.activation(
                                m[:, 0:sp, 0:cw], ps[:, 0:sp, 0:cw],
                                mybir.ActivationFunctionType.Copy, bias=0.0)
                        else:
                            nc.scalar.activation(
                                m[:, 0:1, 0:8], ps[:, 0:1, 0:8],
                                mybir.ActivationFunctionType.Copy, bias=0.0)
                        tr, cr = regs[bi]
                        tv = bass.make_scalar_value(
                            bass.RegisterHandles((tr,)), min_val=0,
                            max_val=8 - sp)
                        cv = bass.make_scalar_value(
                            bass.RegisterHandles((cr,)), min_val=0,
                            max_val=cmax)
                        win = canvas[:, bass.ds(tv, sp), bass.ds(cv, cw)]
                        if "nocopy" not in probes:
                            ceng.tensor_tensor(
                                win, m[:, 0:sp, 0:cw], win, mybir.AluOpType.max)
                        else:
                            win8 = canvas[:, bass.ds(tv, 1), bass.ds(cv, 8)]
                            ceng.tensor_tensor(
                                win8, m[:, 0:1, 0:8], win8,
                                mybir.AluOpType.max)
                    # decode word = v*2^k -> val = 4*v - 6: mask the
                    # mantissa, force the exponent to 4.0, subtract 6
                    if "nodecode" not in probes:
                        cbits = canvas[:, :, :].bitcast(U16)
                        nc.vector.tensor_scalar(
                            cbits, cbits, 0x03FF, 0x4400,
                            mybir.AluOpType.bitwise_and,
                            mybir.AluOpType.bitwise_or)
                        nc.vector.tensor_scalar_add(
                            canvas[:, :, :], canvas[:, :, :], -6.0)
                    out_img = out_d[img].rearrange("(t p) c -> p t c", p=128)
                    if "nostore" not in probes:
                        # fp16 output (host casts to f32): plain HWDGE store
                        nc.sync.dma_start(out_img[:, 0:4, :], canvas[:, 0:4, :])
                        nc.gpsimd.dma_start(out_img[:, 4:8, :], canvas[:, 4:8, :])
                    else:
                        nc.sync.dma_start(out_img[:, 0:1, 0:8],
                                          canvas[:, 0:1, 0:8])

            if loop_reps > 1:
                hints = (mybir.EngineType.DVE, mybir.EngineType.Activation,
                         mybir.EngineType.PE, mybir.EngineType.SP,
                         mybir.EngineType.Pool)
                with tc.For_i(0, loop_reps, 1, hint_engines=hints):
                    pipeline()
            else:
                pipeline()
    nc.compile()
    return nc


_NC_CACHE = []
_MIXES = []


def make_in_maps(masks, rects):
    boxdata, tc, mixes = _host_prep(masks, rects)
    if not _MIXES:
        _MIXES.append(mixes)
    else:
        _MIXES[0] = mixes
    in_maps = []
    for core in range(NCORES):
        gsl = slice(core * IMGS * GROUPS, (core + 1) * IMGS * GROUPS)
        sl = slice(core * NBOX, (core + 1) * NBOX)
        in_maps.append({
            "boxdata": np.ascontiguousarray(boxdata[gsl]),
            "tcoff": np.ascontiguousarray(tc[sl].reshape(1, 2 * NBOX)),
        })
    return in_maps


def kernel(masks, rects, instance_mask):
    in_maps = make_in_maps(masks, rects)
    if not _NC_CACHE or _NC_CACHE[0][0] != _MIXES[0]:
        _NC_CACHE.clear()
        _NC_CACHE.append((_MIXES[0], build_nc(mixes=_MIXES[0])))
    nc = _NC_CACHE[0][1]
    res = run_bass_kernel_spmd(nc, in_maps, list(range(NCORES)))
    out = np.concatenate([np.asarray(res.results[i]["out"]) for i in range(NCORES)],
                         axis=0)
    return out.reshape(B, 1, H, W).astype(np.float32)
